# revision 1
# baseline (speedup 1.0000x reference)
"""Manual-sync (raw bacc) Trainium2 kernel for fused cosine-distance row merge.

Per row i: out[i] = u_i*A[i] + (1-u_i)*B[i], u_i = 0.5 - 0.5*dot_i/scale_i,
scale_i = max(|A_i||B_i|, 1e-8).

Hand-scheduled without TileContext (no Tile tail barrier/sem-clear, ~9.5us of
measured exec in the Tile version). Design:
  - All A/B tiles resident in SBUF (16 MB) -- no recycling; the SP engine
    enqueues every load transfer back-to-back from t=0 on one HWDGE ring.
  - ACT: squares with accum (row sum-squares), then one sqrt over [P,2t]
    with scale=2.0 so sa'*sb' = 2|A||B| (folds the 0.5 of u into r).
  - DVE: product+row-dot via scalar_tensor_tensor accum, 3 tiny stats ops
    (mul, max, recip -> r' = 1/max(2|A||B|, 2e-8)), then a fused custom DVE
    lerp per sub-tile: out = (A-B)*(0.5 - dot*r') + B, in-place into the B
    tile. DVE is software-pipelined one unit ahead (stts of unit i+1 are
    emitted before unit i's stats+lerps) so it never waits on ACT.
  - Stores stream whole units from the merged B tile on the SWDGE ring.
  - Tapered unit schedule [1,1,4,4,4,1,1] sub-tiles: tiny first units start
    compute early; tiny last units keep the final load->store chain short.

Sync: dedicated sem per DMA transfer (+16 on completion; concurrent
transfers' increments interleave, so shared-sem totals prove nothing);
per-engine self-chain sems give same-engine WAW/RAW ordering (the accum
dump tiles are reused every op); cross-engine deps wait on chain counts.
SP ends by waiting on every store sem so the program retires only after
the last byte lands.
"""

import numpy as np

import concourse.bacc as bacc
import concourse.mybir as mybir

N_FULL = 16384
D = 1024
NCORES = 8
ROWS = N_FULL // NCORES  # 2048
P = 128
EPS = 1e-8

F32 = mybir.dt.float32

_LERP2_NAME = "LERP2_MERGE_ANT"


def _get_lerp2_op():
    """Register (idempotently) a custom DVE op:
    out = (in0 - in1) * (imm2 - s0*s1) + in1."""
    from concourse import dve_ops
    from concourse.dve_spec import Spec, Src0, Src1, C0, C1, C2, lower, _has_src1
    from concourse.dve_uop import DveOpSpec

    for op in dve_ops.OPS:
        if op.name == _LERP2_NAME:
            return op

    spec = Spec(
        body=(Src0 - Src1) * (C2 - C0 * C1) + Src1,
        reference=lambda in0, in1, s0, s1, imm2: (in0.astype(np.float32) - in1)
        * (imm2 - s0 * s1)
        + in1,
    )
    row = dve_ops._CUSTOM_DVE_ROW_BASE + len(dve_ops.OPS)
    shas = {}
    for ver in ("v3", "v4"):
        try:
            s = DveOpSpec(
                name=_LERP2_NAME,
                opcode=row,
                uops=lower(spec, ver=ver),
                rd1_en=_has_src1(spec),
            )
            shas[ver] = s.sha(ver)
        except Exception:
            pass
    op = dve_ops.DveOp(_LERP2_NAME, spec, subdim=False, uops_sha=shas)
    dve_ops.OPS.append(op)
    dve_ops.CUSTOM_DVE_SPECS[_LERP2_NAME] = spec
    dve_ops._SUB_OPCODE_FOR_NAME[_LERP2_NAME] = row
    return op


# Unit schedule: (rpp, lo, hi) slices of the rpp-grouped row view. A group
# covers rpp*128 consecutive rows; partition p holds rpp consecutive rows
# concatenated along the free dim, so DMA descriptors are rpp*4KB
# contiguous. t = (hi-lo)*rpp sub-tiles of [128, 1024] per unit.
UNITS = [
    (2, 0, 2),    # rows 0-511     t=4
    (2, 2, 4),    # rows 512-1023  t=4
    (2, 4, 6),    # rows 1024-1535 t=4
    (2, 6, 7),    # rows 1536-1791 t=2
    (1, 14, 15),  # rows 1792-1919 t=1 (short final chains)
    (1, 15, 16),  # rows 1920-2047 t=1
]


class _Chain:
    """Per-engine self-chain: each op waits for the previous op on the same
    engine to retire (Tile emits the same pattern; needed for same-engine
    WAW/RAW on reused tiles like the accum dumps)."""

    def __init__(self, nc, eng, name):
        self.eng = eng
        self.sem = nc.alloc_semaphore(name)
        self.n = 0

    def emit(self, fn):
        if self.n > 0:
            self.eng.wait_ge(self.sem, self.n)
        inst = fn()
        inst.then_inc(self.sem, 1)
        self.n += 1
        return inst


def build_program():
    mul = mybir.AluOpType.mult
    Sq = mybir.ActivationFunctionType.Square
    Sqrt = mybir.ActivationFunctionType.Sqrt
    lerp2 = _get_lerp2_op()

    nc = bacc.Bacc()
    A = nc.declare_dram_parameter("A", [ROWS, D], F32, isOutput=False)
    B = nc.declare_dram_parameter("B", [ROWS, D], F32, isOutput=False)
    O = nc.declare_dram_parameter("out", [ROWS, D], F32, isOutput=True)

    def views(T):
        return {
            r: T[:].rearrange("(g p r) d -> g p (r d)", p=P, r=r)
            for r in (1, 2, 4)
        }

    Av, Bv, Ov = views(A), views(B), views(O)

    def dram_ap(vs, u):
        rpp, lo, hi = u
        return vs[rpp][lo:hi].rearrange("g p f -> p g f")

    n_units = len(UNITS)
    ts = [(hi - lo) * rpp for rpp, lo, hi in UNITS]
    assert sum(ts) == ROWS // P
    rows = []
    for rpp, lo, hi in UNITS:
        rows.extend(range(lo * rpp * P, hi * rpp * P))
    assert sorted(rows) == list(range(ROWS))

    # SBUF tiles: one a/b pair per unit, no recycling. 16 MB total.
    a_tiles, b_tiles = [], []
    for i, u in enumerate(UNITS):
        shape = [P, ts[i] * D]
        a_tiles.append(nc.alloc_sbuf_tensor(f"a{i}", shape, F32))
        b_tiles.append(nc.alloc_sbuf_tensor(f"b{i}", shape, F32))

    def sub_ap(tile, j):
        return tile[:, j * D : (j + 1) * D]

    # stats per unit: ss [P,2t] (A then B sum-squares), s = sqrt(2*ss),
    # dot [P,t], sc/r [P,t]
    ss_t, s_t, dot_t, sc_t, r_t = [], [], [], [], []
    for i, t in enumerate(ts):
        ss_t.append(nc.alloc_sbuf_tensor(f"ss{i}", [P, 2 * t], F32))
        s_t.append(nc.alloc_sbuf_tensor(f"s{i}", [P, 2 * t], F32))
        dot_t.append(nc.alloc_sbuf_tensor(f"dot{i}", [P, t], F32))
        sc_t.append(nc.alloc_sbuf_tensor(f"sc{i}", [P, t], F32))
        r_t.append(nc.alloc_sbuf_tensor(f"r{i}", [P, t], F32))

    pdump = nc.alloc_sbuf_tensor("pdump", [P, D], F32)
    adump = nc.alloc_sbuf_tensor("adump", [P, D], F32)

    ldA = [nc.alloc_semaphore(f"ldA{i}") for i in range(n_units)]
    ldB = [nc.alloc_semaphore(f"ldB{i}") for i in range(n_units)]
    stS = [nc.alloc_semaphore(f"st{i}") for i in range(n_units)]

    act_ch = _Chain(nc, nc.scalar, "act_ch")
    dve_ch = _Chain(nc, nc.vector, "dve_ch")

    # ---- SP: all loads back-to-back on one HWDGE ring (A_i then B_i) ----
    for i, u in enumerate(UNITS):
        nc.sync.dma_start(a_tiles[i][:], dram_ap(Av, u)).then_inc(ldA[i], 16)
        nc.sync.dma_start(b_tiles[i][:], dram_ap(Bv, u)).then_inc(ldB[i], 16)

    # ---- ACT: squares + one scaled sqrt per unit ----
    cq_marks = []  # act_ch count after unit i's sqrt
    for i, u in enumerate(UNITS):
        t = ts[i]
        nc.scalar.wait_ge(ldA[i], 16)
        for j in range(t):
            act_ch.emit(lambda i=i, j=j: nc.scalar.activation(
                adump[:], sub_ap(a_tiles[i], j), Sq,
                accum_out=ss_t[i][:, j : j + 1],
            ))
        nc.scalar.wait_ge(ldB[i], 16)
        for j in range(t):
            act_ch.emit(lambda i=i, j=j, t=t: nc.scalar.activation(
                adump[:], sub_ap(b_tiles[i], j), Sq,
                accum_out=ss_t[i][:, t + j : t + j + 1],
            ))
        # s = sqrt(2*ss): folds the 0.5 of u into r below
        act_ch.emit(lambda i=i: nc.scalar.activation(
            s_t[i][:], ss_t[i][:], Sqrt, scale=2.0
        ))
        cq_marks.append(act_ch.n)

    # ---- DVE: dots, then (pipelined one unit behind) stats + lerps ----
    cm_marks = [0] * n_units  # dve_ch count after unit i's last lerp

    def emit_dots(i):
        nc.vector.wait_ge(ldA[i], 16)
        nc.vector.wait_ge(ldB[i], 16)
        for j in range(ts[i]):
            dve_ch.emit(lambda i=i, j=j: nc.vector.scalar_tensor_tensor(
                pdump[:], sub_ap(a_tiles[i], j), 1.0,
                sub_ap(b_tiles[i], j), mul, mul,
                accum_out=dot_t[i][:, j : j + 1],
            ))

    def emit_finish(i):
        t = ts[i]
        nc.vector.wait_ge(act_ch.sem, cq_marks[i])
        # sc = (2sa^2 * 2sb^2)^0.5 pair product = 2*sa*sb;
        # r = 1/max(sc, 2e-8) = 0.5/max(sa*sb, 1e-8)
        dve_ch.emit(lambda i=i, t=t: nc.vector.tensor_mul(
            sc_t[i][:], s_t[i][:, :t], s_t[i][:, t : 2 * t]))
        dve_ch.emit(lambda i=i: nc.vector.tensor_scalar_max(
            sc_t[i][:], sc_t[i][:], 2 * EPS))
        dve_ch.emit(lambda i=i: nc.vector.reciprocal(r_t[i][:], sc_t[i][:]))
        for j in range(t):
            dve_ch.emit(lambda i=i, j=j: nc.vector._custom_dve(
                lerp2,
                out=sub_ap(b_tiles[i], j),
                in0=sub_ap(a_tiles[i], j),
                in1=sub_ap(b_tiles[i], j),
                s0=dot_t[i][:, j : j + 1],
                s1=r_t[i][:, j : j + 1],
                imm2=0.5,
            ))
        cm_marks[i] = dve_ch.n

    # Sequential per unit: dots then stats+lerps. (A variant that pipelined
    # finish(i-1) after dots(i) regressed badly: it orders lerps -- which
    # gate stores -- behind the NEXT unit's load wait, so late-unit stores
    # all pile up after the last loads.)
    for i in range(n_units):
        emit_dots(i)
        emit_finish(i)

    # ---- Pool: SWDGE stores, deferred until ALL loads are done ----
    # Chip-wide, all 8 cores run this same program; separating the read
    # phase from the write phase avoids HBM read/write mixing when cores
    # drift out of phase (observed as ~7us of idle DMA engines mid-stream
    # on bad draws). Same total bytes, engines stay fed from one queue.
    nc.gpsimd.wait_ge(ldB[n_units - 1], 16)
    for i, u in enumerate(UNITS):
        nc.gpsimd.wait_ge(dve_ch.sem, cm_marks[i])
        nc.gpsimd.dma_start(dram_ap(Ov, u), b_tiles[i][:]).then_inc(stS[i], 16)

    # ---- SP tail: program is done when every store has landed ----
    for i in range(n_units):
        nc.sync.wait_ge(stS[i], 16)

    nc.finalize()
    return nc


_prog_cache = {}


def _get_program():
    key = ("manual", ROWS, D)
    if key not in _prog_cache:
        _prog_cache[key] = build_program()
    return _prog_cache[key]


def kernel(A, B):
    from concourse.bass_utils import run_bass_kernel_spmd

    A = np.asarray(A, dtype=np.float32)
    B = np.asarray(B, dtype=np.float32)
    assert A.shape == (N_FULL, D) and B.shape == (N_FULL, D)

    nc = _get_program()
    in_maps = [
        {
            "A": np.ascontiguousarray(A[i * ROWS : (i + 1) * ROWS]),
            "B": np.ascontiguousarray(B[i * ROWS : (i + 1) * ROWS]),
        }
        for i in range(NCORES)
    ]
    res = run_bass_kernel_spmd(nc, in_maps, list(range(NCORES)))
    return np.concatenate([res.results[i]["out"] for i in range(NCORES)], axis=0)

